# revision 13
# baseline (speedup 1.0000x reference)
r"""Trainium2 Bass kernel for the triangular-DP "MAA layer" problem.

Reference computes, per frame t (T=1024, D=256, L=T+1 counts):
    q_t = (1-p_t) q_{t-1} + p_t shift(q_{t-1})          (Poisson-binomial DP)
    m_t = p_t a m_sh + (1-p_t) m + p_t b q_sh x_t       ([L, D] state)
    out = sum_i m_T[i, :]                               ([D])

Algebraic restructuring: the scan collapses to

    out[d] = sum_t c_t x[t, d],
    c_t    = p_t * I_t,   I_t = int_0^1 prod_{s != t} ((1-p_s) + p_s u) du,

computed with K=64-node Gauss-Legendre quadrature (converged to the f32
noise floor).  With f[t,k] = 1 + p_t (u_k - 1):

    slogw_k = sum_t ln f[t,k] + ln w_k
    c_t     = p_t * sum_k exp(slogw_k - ln f[t,k])
    out     = c^T @ x

Device mapping (t on partitions: 8 chunks of 128; k on free dim, K=64):
  - p arrives as [8,128] (8 DMA packets) and is PE-transposed to [128,8];
    um1 / lnw arrive as single-partition rows and are broadcast across
    partitions with 1-partition PE matmuls (no GpSimd anywhere).
  - lf = Ln(um1*p + 1) on ScalarE (one fused op over all 8 chunks);
    slogw broadcast to [128,K] via a ones[128,128] PE matmul (+ a second
    accumulating matmul adding ln w), so no partition_broadcast needed.
  - e = Exp(slogw - lf_own) with the k-sum fused via activation
    accum_out -> cfin in a single ScalarE op.
  - out_partial = cfin_bf16^T @ x_own_bf16: one PE matmul.

Sharding: core j owns t-chunk j.  Every core computes the full slogw
(needs all of p, which is tiny) but only its own chunk's cfin and only
DMAs its own x chunk (64 KB bf16 instead of 1 MB f32).  The host sums
the 8 partial [D] outputs (the gather/unshard step).  bf16 is used only
for the final contraction (rel err ~2e-3, gate is 2e-2).
"""

import numpy as np

T, D, NCH, P, K = 1024, 256, 8, 128, 64
N_CORES = 8
# pa row layout: [p8 (128) | I8 (8) | um1 (64) | lnw (64)]; um1/lnw only
# occupy row 0 (matmul operands must start at partition 0).
PAW = 128 + 8 + K + K

_CACHE = {}


def _gl_nodes_weights():
    nodes, weights = np.polynomial.legendre.leggauss(K)
    u = (nodes + 1.0) * 0.5
    w = weights * 0.5
    return u, w


def _build_program():
    import concourse.bass as bass
    import concourse.bacc as bacc
    import concourse.mybir as mybir
    import concourse.tile as tile

    f32 = mybir.dt.float32
    bf16 = mybir.dt.bfloat16
    A = mybir.AluOpType
    ACT = mybir.ActivationFunctionType

    nc = bacc.Bacc("TRN2", target_bir_lowering=False, debug=False,
                   num_devices=N_CORES)

    pa_d = nc.dram_tensor("pa", [NCH, PAW], f32, kind="ExternalInput")
    xb_d = nc.dram_tensor("xb", [P, D], bf16, kind="ExternalInput")
    out_d = nc.dram_tensor("out", [1, D], f32, kind="ExternalOutput")

    with tile.TileContext(nc) as tc:
        with (
            tc.tile_pool(name="sb", bufs=1) as sb,
            tc.tile_pool(name="ps", bufs=1, space=bass.MemorySpace.PSUM) as ps,
        ):
            pa = sb.tile([NCH, PAW], f32, tag="pa")
            xb = sb.tile([P, D], bf16, tag="xb")
            ones128 = sb.tile([P, P], f32, tag="ones128")
            fm1 = sb.tile([P, NCH * K], f32, tag="fm1")
            lfbig = sb.tile([P, NCH * K], f32, tag="lfbig")
            lfsum = sb.tile([P, K], f32, tag="lfsum")
            esub = sb.tile([P, K], f32, tag="esub")
            junk = sb.tile([P, K], f32, tag="junk")
            pcol_sb = sb.tile([P, NCH], f32, tag="pcol_sb")
            cfin8 = sb.tile([P, 1], f32, tag="cfin8")
            cfinb = sb.tile([P, 1], bf16, tag="cfinb")
            outsb = sb.tile([1, D], f32, tag="outsb")

            pcol = ps.tile([P, NCH], f32, tag="pcol")
            um1bc = ps.tile([P, K], f32, tag="um1bc")
            slogw = ps.tile([P, K], f32, tag="slogw")
            outp = ps.tile([1, D], f32, tag="outp")

            # DMAs on both HWDGE queues in parallel: the tiny scalars on
            # SP, the bulk x chunk on Activation.
            nc.sync.dma_start(pa[:], pa_d[:])
            nc.scalar.dma_start(xb[:], xb_d[:])

            # Early, dependency-free work during the DMA shadow: the
            # ones matrix, and a dummy Ln to pull the activation-table
            # load off the critical path.
            nc.vector.memset(ones128[:], 1.0)
            nc.scalar.activation(junk[0:1, 0:1], ones128[0:1, 0:1], ACT.Ln)

            p8v = pa[0:NCH, 0:P]
            iden = pa[0:NCH, P:P + NCH]
            um1r = pa[0:1, P + NCH:P + NCH + K]
            lnwr = pa[0:1, P + NCH + K:P + NCH + 2 * K]

            # pcol[t, c] = p chunk c (PE transpose), um1 broadcast to all
            # partitions via a 1-partition matmul.
            nc.tensor.transpose(pcol[:], p8v, iden)
            nc.tensor.matmul(um1bc[:], ones128[0:1, :], um1r,
                             start=True, stop=True)
            # DVE reads at most one PSUM operand per op: stage pcol in SBUF
            # (free: DVE is idle until the um1 broadcast lands anyway).
            nc.vector.tensor_copy(pcol_sb[:], pcol[:])

            # fm1[t,(c,k)] = p[c,t] * um1[k]; lf = Ln(fm1 + 1)
            nc.vector.tensor_tensor(
                fm1.rearrange("p (c k) -> p c k", c=NCH),
                um1bc.unsqueeze(1).broadcast_to([P, NCH, K]),
                pcol_sb.unsqueeze(2).broadcast_to([P, NCH, K]),
                op=A.mult)
            nc.scalar.activation(lfbig[:], fm1[:], ACT.Ln, bias=1.0)
            # Dummy Exp right after the Ln in ScalarE program order: its
            # hoisted table load has no data waits, so the Ln->Exp table
            # switch happens in the shadow of the DVE reduce + PE matmul
            # instead of on the critical path before the real Exp.
            nc.scalar.activation(junk[0:1, 1:2], ones128[0:1, 0:1], ACT.Exp)

            # slogw[*, k] = sum_t ln f[t,k] + ln w_k, broadcast to all
            # partitions via a ones[128,128] matmul.  ln w is folded in by
            # adding it to partition 0 of lfsum before the partition-sum.
            nc.vector.tensor_reduce(
                lfsum[:], lfbig.rearrange("p (c k) -> p k c", c=NCH),
                axis=mybir.AxisListType.X, op=A.add)
            nc.vector.tensor_tensor(lfsum[0:1, :], lfsum[0:1, :], lnwr,
                                    op=A.add)
            nc.tensor.matmul(slogw[:], ones128[:], lfsum[:],
                             start=True, stop=True)

            # cfin8[t] = sum_k exp(slogw[k] - lf_own[t,k]) in ONE ScalarE
            # op via accum_out; chunk 0 is the core's own chunk.
            nc.vector.tensor_tensor(esub[:], slogw[:], lfbig[:, 0:K],
                                    op=A.subtract)
            nc.scalar.activation(junk[:], esub[:], ACT.Exp,
                                 accum_out=cfin8[:])

            # cfin = p_own * cfin8, cast to bf16 for the PE contraction.
            nc.vector.tensor_tensor(cfinb[:], cfin8[:], pcol_sb[:, 0:1],
                                    op=A.mult)

            # partial out[d] = sum_t cfin[t] x_own[t, d]
            nc.tensor.matmul(outp[:], cfinb[:], xb[:], start=True, stop=True)
            nc.vector.tensor_copy(outsb[:], outp[:])
            nc.sync.dma_start(out_d[:], outsb[:])

    nc.compile()
    return nc


def _make_in_maps(p, x):
    p1 = np.ascontiguousarray(np.asarray(p, dtype=np.float32)).reshape(T)
    x3 = np.ascontiguousarray(np.asarray(x, dtype=np.float32)).reshape(
        NCH, P, D)
    u, w = _gl_nodes_weights()
    um1 = (u - 1.0).astype(np.float32)
    lnw = np.log(w).astype(np.float32)
    p8_all = p1.reshape(NCH, P)

    from ml_dtypes import bfloat16

    in_maps = []
    for j in range(N_CORES):
        perm = [j] + [c for c in range(NCH) if c != j]
        pa = np.zeros((NCH, PAW), np.float32)
        pa[0:NCH, 0:P] = p8_all[perm]
        pa[0:NCH, P:P + NCH] = np.eye(NCH, dtype=np.float32)
        pa[0, P + NCH:P + NCH + K] = um1
        pa[0, P + NCH + K:P + NCH + 2 * K] = lnw
        xbj = np.ascontiguousarray(x3[j]).astype(bfloat16)
        in_maps.append({"pa": np.ascontiguousarray(pa), "xb": xbj})
    return in_maps


def _run(p, x, trace=False, tmpdir=None):
    from concourse.bass_utils import run_bass_kernel_spmd

    if "nc" not in _CACHE:
        _CACHE["nc"] = _build_program()
    nc = _CACHE["nc"]
    in_maps = _make_in_maps(p, x)
    res = run_bass_kernel_spmd(nc, in_maps, list(range(N_CORES)),
                               trace=trace, tmpdir=tmpdir)
    out = np.zeros(D, np.float64)
    for j in range(N_CORES):
        out += np.asarray(res.results[j]["out"], dtype=np.float64).reshape(D)
    return out.astype(np.float32), res


def kernel(p, x):
    out, _ = _run(p, x, trace=False)
    return out


# revision 14
# speedup vs baseline: 1.1809x; 1.1809x over previous
r"""Trainium2 Bass kernel for the triangular-DP "MAA layer" problem.

Reference computes, per frame t (T=1024, D=256, L=T+1 counts):
    q_t = (1-p_t) q_{t-1} + p_t shift(q_{t-1})          (Poisson-binomial DP)
    m_t = p_t a m_sh + (1-p_t) m + p_t b q_sh x_t       ([L, D] state)
    out = sum_i m_T[i, :]                               ([D])

Algebraic restructuring: the scan collapses to

    out[d] = sum_t c_t x[t, d],
    c_t    = p_t * I_t,   I_t = int_0^1 prod_{s != t} ((1-p_s) + p_s u) du,

computed with K=64-node Gauss-Legendre quadrature (converged to the f32
noise floor).  With f[t,k] = 1 + p_t (u_k - 1):

    slogw_k = sum_t ln f[t,k] + ln w_k
    c_t     = p_t * sum_k exp(slogw_k - ln f[t,k])
    out     = c^T @ x

Device mapping (t on partitions: 8 chunks of 128; k on free dim, K=64):
  - p arrives as [8,128] (8 DMA packets) and is PE-transposed to [128,8];
    um1 / lnw arrive as single-partition rows and are broadcast across
    partitions with 1-partition PE matmuls (no GpSimd anywhere).
  - lf = Ln(um1*p + 1) on ScalarE (one fused op over all 8 chunks);
    slogw broadcast to [128,K] via a ones[128,128] PE matmul (+ a second
    accumulating matmul adding ln w), so no partition_broadcast needed.
  - e = Exp(slogw - lf_own) with the k-sum fused via activation
    accum_out -> cfin in a single ScalarE op.
  - out_partial = cfin_bf16^T @ x_own_bf16: one PE matmul.

Sharding: core j owns t-chunk j.  Every core computes the full slogw
(needs all of p, which is tiny) but only its own chunk's cfin and only
DMAs its own x chunk (64 KB bf16 instead of 1 MB f32).  The host sums
the 8 partial [D] outputs (the gather/unshard step).  bf16 is used only
for the final contraction (rel err ~2e-3, gate is 2e-2).
"""

import numpy as np

T, D, NCH, P, K = 1024, 256, 8, 128, 64
N_CORES = 8
# pa row layout: [p8 (128) | I8 (8) | um1 (64) | lnw (64)]; um1/lnw only
# occupy row 0 (matmul operands must start at partition 0).
PAW = 128 + 8 + K + K

_CACHE = {}


def _gl_nodes_weights():
    nodes, weights = np.polynomial.legendre.leggauss(K)
    u = (nodes + 1.0) * 0.5
    w = weights * 0.5
    return u, w


def _build_program():
    import concourse.bass as bass
    import concourse.bacc as bacc
    import concourse.mybir as mybir
    import concourse.tile as tile

    f32 = mybir.dt.float32
    bf16 = mybir.dt.bfloat16
    A = mybir.AluOpType
    ACT = mybir.ActivationFunctionType

    nc = bacc.Bacc("TRN2", target_bir_lowering=False, debug=False,
                   num_devices=N_CORES)

    pa_d = nc.dram_tensor("pa", [NCH, PAW], f32, kind="ExternalInput")
    xb_d = nc.dram_tensor("xb", [P, D], bf16, kind="ExternalInput")
    out_d = nc.dram_tensor("out", [1, D], f32, kind="ExternalOutput")

    with tile.TileContext(nc) as tc:
        with (
            tc.tile_pool(name="sb", bufs=1) as sb,
            tc.tile_pool(name="ps", bufs=1, space=bass.MemorySpace.PSUM) as ps,
        ):
            pa = sb.tile([NCH, PAW], f32, tag="pa")
            xb = sb.tile([P, D], bf16, tag="xb")
            ones128 = sb.tile([P, P], f32, tag="ones128")
            fm1 = sb.tile([P, NCH * K], f32, tag="fm1")
            lfbig = sb.tile([P, NCH * K], f32, tag="lfbig")
            lfsum = sb.tile([P, K], f32, tag="lfsum")
            esub = sb.tile([P, K], f32, tag="esub")
            junk = sb.tile([P, K], f32, tag="junk")
            pcol_sb = sb.tile([P, NCH], f32, tag="pcol_sb")
            cfin8 = sb.tile([P, 1], f32, tag="cfin8")
            cfinb = sb.tile([P, 1], bf16, tag="cfinb")
            outsb = sb.tile([1, D], f32, tag="outsb")

            pcol = ps.tile([P, NCH], f32, tag="pcol")
            um1bc = ps.tile([P, K], f32, tag="um1bc")
            slogw = ps.tile([P, K], f32, tag="slogw")
            outp = ps.tile([1, D], f32, tag="outp")

            # DMAs on both HWDGE queues in parallel: the tiny scalars on
            # SP, the bulk x chunk on Activation.
            nc.sync.dma_start(pa[:], pa_d[:])
            nc.scalar.dma_start(xb[:], xb_d[:])

            # Early, dependency-free work during the DMA shadow: the
            # ones matrix, and a dummy Ln to pull the activation-table
            # load off the critical path.
            nc.vector.memset(ones128[:], 1.0)
            nc.scalar.activation(junk[0:1, 0:1], ones128[0:1, 0:1], ACT.Ln)

            p8v = pa[0:NCH, 0:P]
            iden = pa[0:NCH, P:P + NCH]
            um1r = pa[0:1, P + NCH:P + NCH + K]
            lnwr = pa[0:1, P + NCH + K:P + NCH + 2 * K]

            # pcol[t, c] = p chunk c (PE transpose), um1 broadcast to all
            # partitions via a 1-partition matmul.
            nc.tensor.transpose(pcol[:], p8v, iden)
            nc.tensor.matmul(um1bc[:], ones128[0:1, :], um1r,
                             start=True, stop=True)
            # DVE reads at most one PSUM operand per op: stage pcol in SBUF
            # (free: DVE is idle until the um1 broadcast lands anyway).
            nc.vector.tensor_copy(pcol_sb[:], pcol[:])

            # fm1[t,(c,k)] = p[c,t] * um1[k]; lf = Ln(fm1 + 1)
            nc.vector.tensor_tensor(
                fm1.rearrange("p (c k) -> p c k", c=NCH),
                um1bc.unsqueeze(1).broadcast_to([P, NCH, K]),
                pcol_sb.unsqueeze(2).broadcast_to([P, NCH, K]),
                op=A.mult)
            nc.scalar.activation(lfbig[:], fm1[:], ACT.Ln, bias=1.0)
            # Dummy Exp reading lfbig so the scheduler pins it right after
            # the Ln: the Ln->Exp table switch then runs in the shadow of
            # the DVE reduce + PE matmul instead of stalling the real Exp
            # (whose table load would otherwise inherit esub's waits).
            nc.scalar.activation(junk[0:1, 1:2], lfbig[0:1, 0:1], ACT.Exp)

            # slogw[*, k] = sum_t ln f[t,k] + ln w_k, broadcast to all
            # partitions via a ones[128,128] matmul.  ln w is folded in by
            # adding it to partition 0 of lfsum before the partition-sum.
            nc.vector.tensor_reduce(
                lfsum[:], lfbig.rearrange("p (c k) -> p k c", c=NCH),
                axis=mybir.AxisListType.X, op=A.add)
            nc.vector.tensor_tensor(lfsum[0:1, :], lfsum[0:1, :], lnwr,
                                    op=A.add)
            nc.tensor.matmul(slogw[:], ones128[:], lfsum[:],
                             start=True, stop=True)

            # cfin8[t] = sum_k exp(slogw[k] - lf_own[t,k]) in ONE ScalarE
            # op via accum_out; chunk 0 is the core's own chunk.
            nc.vector.tensor_tensor(esub[:], slogw[:], lfbig[:, 0:K],
                                    op=A.subtract)
            nc.scalar.activation(junk[:], esub[:], ACT.Exp,
                                 accum_out=cfin8[:])

            # cfin = p_own * cfin8, cast to bf16 for the PE contraction.
            nc.vector.tensor_tensor(cfinb[:], cfin8[:], pcol_sb[:, 0:1],
                                    op=A.mult)

            # partial out[d] = sum_t cfin[t] x_own[t, d]
            nc.tensor.matmul(outp[:], cfinb[:], xb[:], start=True, stop=True)
            nc.vector.tensor_copy(outsb[:], outp[:])
            nc.sync.dma_start(out_d[:], outsb[:])

    nc.compile()
    return nc


def _make_in_maps(p, x):
    p1 = np.ascontiguousarray(np.asarray(p, dtype=np.float32)).reshape(T)
    x3 = np.ascontiguousarray(np.asarray(x, dtype=np.float32)).reshape(
        NCH, P, D)
    u, w = _gl_nodes_weights()
    um1 = (u - 1.0).astype(np.float32)
    lnw = np.log(w).astype(np.float32)
    p8_all = p1.reshape(NCH, P)

    from ml_dtypes import bfloat16

    in_maps = []
    for j in range(N_CORES):
        perm = [j] + [c for c in range(NCH) if c != j]
        pa = np.zeros((NCH, PAW), np.float32)
        pa[0:NCH, 0:P] = p8_all[perm]
        pa[0:NCH, P:P + NCH] = np.eye(NCH, dtype=np.float32)
        pa[0, P + NCH:P + NCH + K] = um1
        pa[0, P + NCH + K:P + NCH + 2 * K] = lnw
        xbj = np.ascontiguousarray(x3[j]).astype(bfloat16)
        in_maps.append({"pa": np.ascontiguousarray(pa), "xb": xbj})
    return in_maps


def _run(p, x, trace=False, tmpdir=None):
    from concourse.bass_utils import run_bass_kernel_spmd

    if "nc" not in _CACHE:
        _CACHE["nc"] = _build_program()
    nc = _CACHE["nc"]
    in_maps = _make_in_maps(p, x)
    res = run_bass_kernel_spmd(nc, in_maps, list(range(N_CORES)),
                               trace=trace, tmpdir=tmpdir)
    out = np.zeros(D, np.float64)
    for j in range(N_CORES):
        out += np.asarray(res.results[j]["out"], dtype=np.float64).reshape(D)
    return out.astype(np.float32), res


def kernel(p, x):
    out, _ = _run(p, x, trace=False)
    return out


# revision 19
# speedup vs baseline: 1.2333x; 1.0444x over previous
r"""Trainium2 Bass kernel for the triangular-DP "MAA layer" problem.

Reference computes, per frame t (T=1024, D=256, L=T+1 counts):
    q_t = (1-p_t) q_{t-1} + p_t shift(q_{t-1})          (Poisson-binomial DP)
    m_t = p_t a m_sh + (1-p_t) m + p_t b q_sh x_t       ([L, D] state)
    out = sum_i m_T[i, :]                               ([D])

Algebraic restructuring: the scan collapses to

    out[d] = sum_t c_t x[t, d],
    c_t    = p_t * I_t,   I_t = int_0^1 prod_{s != t} ((1-p_s) + p_s u) du,

computed with K=64-node Gauss-Legendre quadrature (converged to the f32
noise floor).  With f[t,k] = 1 + p_t (u_k - 1):

    slogw_k = sum_t ln f[t,k] + ln w_k
    c_t     = p_t * sum_k exp(slogw_k - ln f[t,k])
    out     = c^T @ x

Device mapping (t on partitions: 8 chunks of 128; k on free dim, K=64):
  - p arrives as [8,128] (8 DMA packets) and is PE-transposed to [128,8];
    um1 / lnw arrive as single-partition rows and are broadcast across
    partitions with 1-partition PE matmuls (no GpSimd anywhere).
  - lf = Ln(um1*p + 1) on ScalarE (one fused op over all 8 chunks);
    slogw broadcast to [128,K] via a ones[128,128] PE matmul (+ a second
    accumulating matmul adding ln w), so no partition_broadcast needed.
  - e = Exp(slogw - lf_own) with the k-sum fused via activation
    accum_out -> cfin in a single ScalarE op.
  - out_partial = cfin_bf16^T @ x_own_bf16: one PE matmul.

Sharding: core j owns t-chunk j.  Every core computes the full slogw
(needs all of p, which is tiny) but only its own chunk's cfin and only
DMAs its own x chunk (64 KB bf16 instead of 1 MB f32).  The host sums
the 8 partial [D] outputs (the gather/unshard step).  bf16 is used only
for the final contraction (rel err ~2e-3, gate is 2e-2).
"""

import numpy as np

T, D, NCH, P, K = 1024, 256, 8, 128, 32
N_CORES = 8
# pa row layout: [p8 (128) | I8 (8) | um1 hi/lo bf16 packed (K f32 slots)
# | lnw (K f32)]; the scalar rows only occupy row 0 (matmul operands must
# start at partition 0).  K=32 Gauss-Legendre is converged to ~2.5e-3
# (gate is 2e-2); um1 is split hi+lo bf16 so the partition-broadcast can
# use two exact accumulating bf16 matmuls instead of a slow fp32r one.
PAW = 128 + 8 + K + K
UM1_C = 128 + 8
LNW_C = UM1_C + K

_CACHE = {}


def _gl_nodes_weights():
    nodes, weights = np.polynomial.legendre.leggauss(K)
    u = (nodes + 1.0) * 0.5
    w = weights * 0.5
    return u, w


def _build_program():
    import concourse.bass as bass
    import concourse.bacc as bacc
    import concourse.mybir as mybir
    import concourse.tile as tile

    f32 = mybir.dt.float32
    bf16 = mybir.dt.bfloat16
    A = mybir.AluOpType
    ACT = mybir.ActivationFunctionType

    nc = bacc.Bacc("TRN2", target_bir_lowering=False, debug=False,
                   num_devices=N_CORES)

    pa_d = nc.dram_tensor("pa", [NCH, PAW], f32, kind="ExternalInput")
    xb_d = nc.dram_tensor("xb", [P, D], bf16, kind="ExternalInput")
    out_d = nc.dram_tensor("out", [1, D], f32, kind="ExternalOutput")

    with tile.TileContext(nc) as tc:
        with (
            tc.tile_pool(name="sb", bufs=1) as sb,
            tc.tile_pool(name="ps", bufs=1, space=bass.MemorySpace.PSUM) as ps,
        ):
            pa = sb.tile([NCH, PAW], f32, tag="pa")
            xb = sb.tile([P, D], bf16, tag="xb")
            ones128 = sb.tile([P, P], f32, tag="ones128")
            onesrow_bf = sb.tile([1, P], bf16, tag="onesrow_bf")
            fm1 = sb.tile([P, NCH * K], f32, tag="fm1")
            lfbig = sb.tile([P, NCH * K], f32, tag="lfbig")
            lfsum = sb.tile([P, K], f32, tag="lfsum")
            esub = sb.tile([P, K], f32, tag="esub")
            junk = sb.tile([P, K], f32, tag="junk")
            pcol_sb = sb.tile([P, NCH], f32, tag="pcol_sb")
            cfin8 = sb.tile([P, 1], f32, tag="cfin8")
            cfinb = sb.tile([P, 1], bf16, tag="cfinb")
            outsb = sb.tile([1, D], f32, tag="outsb")

            pcol = ps.tile([P, NCH], f32, tag="pcol")
            um1bc = ps.tile([P, K], f32, tag="um1bc")
            slogw = ps.tile([P, K], f32, tag="slogw")
            outp = ps.tile([1, D], f32, tag="outp")

            # DMAs on both HWDGE queues in parallel: the tiny scalars on
            # SP, the bulk x chunk on Activation.
            nc.sync.dma_start(pa[:], pa_d[:])
            nc.scalar.dma_start(xb[:], xb_d[:])

            # Early, dependency-free work during the DMA shadow: the
            # ones matrix, and a dummy Ln to pull the activation-table
            # load off the critical path.
            nc.vector.memset(ones128[:], 1.0)
            nc.vector.memset(onesrow_bf[:], 1.0)
            nc.scalar.activation(junk[0:1, 0:1], ones128[0:1, 0:1], ACT.Ln)

            p8v = pa[0:NCH, 0:P]
            iden = pa[0:NCH, P:P + NCH]
            um1hl = pa[0:1, UM1_C:UM1_C + K].bitcast(bf16)  # [1, 2K] bf16
            lnwr = pa[0:1, LNW_C:LNW_C + K]

            # pcol[t, c] = p chunk c (PE transpose); um1 broadcast to all
            # partitions via two exact hi+lo bf16 1-partition matmuls.
            nc.tensor.transpose(pcol[:], p8v, iden)
            nc.tensor.matmul(um1bc[:], onesrow_bf[:], um1hl[:, 0:K],
                             start=True, stop=False)
            nc.tensor.matmul(um1bc[:], onesrow_bf[:], um1hl[:, K:2 * K],
                             start=False, stop=True)
            # DVE reads at most one PSUM operand per op: stage pcol in SBUF
            # (free: DVE is idle until the um1 broadcast lands anyway).
            nc.vector.tensor_copy(pcol_sb[:], pcol[:])

            # fm1[t,(c,k)] = p[c,t] * um1[k]; lf = Ln(fm1 + 1)
            nc.vector.tensor_tensor(
                fm1.rearrange("p (c k) -> p c k", c=NCH),
                um1bc.unsqueeze(1).broadcast_to([P, NCH, K]),
                pcol_sb.unsqueeze(2).broadcast_to([P, NCH, K]),
                op=A.mult)
            nc.scalar.activation(lfbig[:], fm1[:], ACT.Ln, bias=1.0)
            # Dummy Exp reading lfbig so the scheduler pins it right after
            # the Ln: the Ln->Exp table switch then runs in the shadow of
            # the DVE reduce + PE matmul instead of stalling the real Exp
            # (whose table load would otherwise inherit esub's waits).
            nc.scalar.activation(junk[0:1, 1:2], lfbig[0:1, 0:1], ACT.Exp)

            # slogw[*, k] = sum_t ln f[t,k] + ln w_k, broadcast to all
            # partitions via a ones[128,128] matmul.  ln w is folded in by
            # adding it to partition 0 of lfsum before the partition-sum.
            nc.vector.tensor_reduce(
                lfsum[:], lfbig.rearrange("p (c k) -> p k c", c=NCH),
                axis=mybir.AxisListType.X, op=A.add)
            nc.vector.tensor_tensor(lfsum[0:1, :], lfsum[0:1, :], lnwr,
                                    op=A.add)
            nc.tensor.matmul(slogw[:], ones128[:], lfsum[:],
                             start=True, stop=True)

            # cfin8[t] = sum_k exp(slogw[k] - lf_own[t,k]) in ONE ScalarE
            # op via accum_out; chunk 0 is the core's own chunk.
            nc.vector.tensor_tensor(esub[:], slogw[:], lfbig[:, 0:K],
                                    op=A.subtract)
            nc.scalar.activation(junk[:], esub[:], ACT.Exp,
                                 accum_out=cfin8[:])

            # cfin = p_own * cfin8, cast to bf16 for the PE contraction.
            nc.vector.tensor_tensor(cfinb[:], cfin8[:], pcol_sb[:, 0:1],
                                    op=A.mult)

            # partial out[d] = sum_t cfin[t] x_own[t, d]
            nc.tensor.matmul(outp[:], cfinb[:], xb[:], start=True, stop=True)
            nc.vector.tensor_copy(outsb[:], outp[:])
            nc.sync.dma_start(out_d[:], outsb[:])

    nc.compile()
    return nc


def _make_in_maps(p, x):
    p1 = np.ascontiguousarray(np.asarray(p, dtype=np.float32)).reshape(T)
    x3 = np.ascontiguousarray(np.asarray(x, dtype=np.float32)).reshape(
        NCH, P, D)
    u, w = _gl_nodes_weights()
    um1 = (u - 1.0).astype(np.float32)
    lnw = np.log(w).astype(np.float32)
    p8_all = p1.reshape(NCH, P)

    from ml_dtypes import bfloat16

    um1_hi = um1.astype(bfloat16)
    um1_lo = (um1 - um1_hi.astype(np.float32)).astype(bfloat16)
    um1_packed = np.concatenate([um1_hi, um1_lo]).view(np.float32)  # [K]

    in_maps = []
    for j in range(N_CORES):
        perm = [j] + [c for c in range(NCH) if c != j]
        pa = np.zeros((NCH, PAW), np.float32)
        pa[0:NCH, 0:P] = p8_all[perm]
        pa[0:NCH, P:P + NCH] = np.eye(NCH, dtype=np.float32)
        pa[0, UM1_C:UM1_C + K] = um1_packed
        pa[0, LNW_C:LNW_C + K] = lnw
        xbj = np.ascontiguousarray(x3[j]).astype(bfloat16)
        in_maps.append({"pa": np.ascontiguousarray(pa), "xb": xbj})
    return in_maps


def _run(p, x, trace=False, tmpdir=None):
    from concourse.bass_utils import run_bass_kernel_spmd

    if "nc" not in _CACHE:
        _CACHE["nc"] = _build_program()
    nc = _CACHE["nc"]
    in_maps = _make_in_maps(p, x)
    res = run_bass_kernel_spmd(nc, in_maps, list(range(N_CORES)),
                               trace=trace, tmpdir=tmpdir)
    out = np.zeros(D, np.float64)
    for j in range(N_CORES):
        out += np.asarray(res.results[j]["out"], dtype=np.float64).reshape(D)
    return out.astype(np.float32), res


def kernel(p, x):
    out, _ = _run(p, x, trace=False)
    return out
